# revision 10
# baseline (speedup 1.0000x reference)
"""Trainium2 Bass kernel for the CRA relation module.

Math: the reference computes, per sample,
    phi_x = relu((x@W1+b1)*g1+be1), phi_y likewise,  cat_phi = [phi_x; phi_y]
    A = cat_phi cat_phi^T (symmetric!),  R = [A | A^T] = [A | A]
    W = (cat_phi@W3+b3)@W5a + (R@W4+b4)@W5b + b5
    out = x * W[:196] + y * W[196:]
Because A is symmetric and everything after A is linear into a scalar per
token, the relation pipeline collapses to per-sample matvecs:
    u3 = W3@W5a, u4 = W4@W5b, z = u4[:392]+u4[392:], c0 = b3@W5a+b4@W5b+b5
    s  = u3 + phi_x^T z[:196] + phi_y^T z[196:]          (768-vector)
    out = x*(phi_x@s + c0) + y*(phi_y@s + c0)
The device only computes the two 768x768 "1x1 conv" matmuls (the dominant
cost), one fused multiply-reduce, and one matvec per stream. Everything is
data-parallel over the batch: 16 samples per core on 8 cores.

Device layout: feature-major ("transposed") so the contraction dim (cin)
sits on SBUF partitions. The host packs x into [group, 128, 6*392] where
each 392-column block holds [x_a | x_b] for one cin tile of two samples,
so one DMA per group is fully contiguous.
"""

import numpy as np
from contextlib import ExitStack

import concourse.bass as bass
import concourse.tile as tile
import concourse.mybir as mybir
from concourse.bass_utils import run_bass_kernel_spmd

F32 = mybir.dt.float32
F16 = mybir.dt.float16
ALU = mybir.AluOpType
ACTF = mybir.ActivationFunctionType

B, N, C = 128, 196, 768
NCORES = 8
S = B // NCORES          # 16 samples per core
G = 2                    # samples per weight pass (moving N = 392 <= 512 fp32)
NG = S // G              # 8 groups per core
DT = C // 128            # 6 feature tiles
W2T = 2 * N              # 392


def build_bass(c0: float) -> bass.Bass:
    nc = bass.Bass()
    xg_d = nc.declare_dram_parameter("xg", [NG, 128, DT * W2T], F16, isOutput=False)
    yg_d = nc.declare_dram_parameter("yg", [NG, 128, DT * W2T], F16, isOutput=False)
    w1_d = nc.declare_dram_parameter("w1", [C, C], F16, isOutput=False)
    w2_d = nc.declare_dram_parameter("w2", [C, C], F16, isOutput=False)
    zb_d = nc.declare_dram_parameter("zb", [128, W2T], F16, isOutput=False)
    u3_d = nc.declare_dram_parameter("u3", [128, DT], F32, isOutput=False)
    b1_d = nc.declare_dram_parameter("b1", [128, DT], F32, isOutput=False)
    b2_d = nc.declare_dram_parameter("b2", [128, DT], F32, isOutput=False)
    out_d = nc.declare_dram_parameter("out", [S, 128, DT * N], F16, isOutput=True)

    with tile.TileContext(nc) as tc, ExitStack() as ctx:
        const = ctx.enter_context(tc.tile_pool(name="const", bufs=1))

        w1_sb, w2_sb = [], []
        for k in range(DT):
            t1 = const.tile([128, C], F16, tag=f"w1_{k}")
            nc.sync.dma_start(out=t1[:], in_=w1_d[k * 128:(k + 1) * 128, :])
            w1_sb.append(t1)
            t2 = const.tile([128, C], F16, tag=f"w2_{k}")
            nc.sync.dma_start(out=t2[:], in_=w2_d[k * 128:(k + 1) * 128, :])
            w2_sb.append(t2)

        zb = const.tile([128, W2T], F16, tag="zb")
        nc.sync.dma_start(out=zb[:], in_=zb_d[:, :])
        u3 = const.tile([128, DT], F32, tag="u3")
        nc.sync.dma_start(out=u3[:], in_=u3_d[:, :])
        b1t = const.tile([128, DT], F32, tag="b1")
        nc.sync.dma_start(out=b1t[:], in_=b1_d[:, :])
        b2t = const.tile([128, DT], F32, tag="b2")
        nc.sync.dma_start(out=b2t[:], in_=b2_d[:, :])
        ones = const.tile([128, 128], F16, tag="ones")
        nc.vector.memset(ones[:], 1.0)
        # Absorb the bias-tile DMA deps into ACT program order now, so the
        # relu evictions later only ever wait on the PE semaphore (the ISA
        # Activation descriptor holds a single sync-wait).
        warm1 = const.tile([128, 1], F32, tag="warm1")
        warm2 = const.tile([128, 1], F32, tag="warm2")
        nc.scalar.activation(warm1[:], b1t[:, 0:1], ACTF.Copy)
        nc.scalar.activation(warm2[:], b2t[:, 0:1], ACTF.Copy)

        xin = ctx.enter_context(tc.tile_pool(name="xin", bufs=3))
        phip = ctx.enter_context(tc.tile_pool(name="phi", bufs=3))
        sp = ctx.enter_context(tc.tile_pool(name="sp", bufs=3))
        op = ctx.enter_context(tc.tile_pool(name="op", bufs=2))
        ps = ctx.enter_context(tc.tile_pool(name="ps", bufs=2, space="PSUM"))

        def emit_mains(g):
            xg = xin.tile([128, DT * W2T], F16, tag="xg", name="xg")
            yg = xin.tile([128, DT * W2T], F16, tag="yg", name="yg")
            nc.sync.dma_start(out=xg[:], in_=xg_d[g])
            nc.sync.dma_start(out=yg[:], in_=yg_d[g])
            # phixy[i][d]: [128, 392] = [phi_x | phi_y] of sample (2g+i), tile d
            phixy = [[phip.tile([128, W2T], F16, tag=f"phi_{i}_{d}",
                                name=f"phi_{i}_{d}") for d in range(DT)]
                     for i in range(G)]
            for d in range(DT):
                psx = ps.tile([128, W2T], F32, tag="psx", name="psx", bufs=3)
                psy = ps.tile([128, W2T], F32, tag="psy", name="psy", bufs=3)
                for k in range(DT):
                    nc.tensor.matmul(
                        psx[:], w1_sb[k][:, d * 128:(d + 1) * 128],
                        xg[:, k * W2T:(k + 1) * W2T],
                        start=(k == 0), stop=(k == DT - 1))
                for k in range(DT):
                    nc.tensor.matmul(
                        psy[:], w2_sb[k][:, d * 128:(d + 1) * 128],
                        yg[:, k * W2T:(k + 1) * W2T],
                        start=(k == 0), stop=(k == DT - 1))
                for i in range(G):
                    nc.scalar.activation(phixy[i][d][:, 0:N], psx[:, i * N:(i + 1) * N],
                                         ACTF.Relu, bias=b1t[:, d:d + 1])
                    nc.scalar.activation(phixy[i][d][:, N:W2T], psy[:, i * N:(i + 1) * N],
                                         ACTF.Relu, bias=b2t[:, d:d + 1])
            return xg, yg, phixy

        def emit_tail(g, xg, yg, phixy):
            for i in range(G):
                # s = u3 + phi_x^T zx + phi_y^T zy  (per feature tile d)
                t_sb = sp.tile([128, DT], F32, tag=f"t_{i}", name=f"t_{i}")
                s_sb = sp.tile([128, DT], F32, tag=f"s_{i}", name=f"s_{i}")
                for d in range(DT):
                    scr = sp.tile([128, W2T], F16, tag="ttr_scr", name="scr")
                    nc.vector.scalar_tensor_tensor(
                        out=scr[:], in0=phixy[i][d][:], scalar=1.0, in1=zb[:],
                        op0=ALU.mult, op1=ALU.mult,
                        accum_out=t_sb[:, d:d + 1])
                    nc.vector.tensor_scalar_add(
                        s_sb[:, d:d + 1], t_sb[:, d:d + 1], u3[:, d:d + 1])
                # wxy[tok] = phi @ s on all 128 partitions via bcast lhsT
                psw = ps.tile([128, W2T], F32, tag="psw", name="psw")
                sbc = [sp.tile([128, 128], F16, tag=f"sbc_{d}", name=f"sbc_{d}")
                       for d in range(DT)]
                for d in range(DT):
                    nc.scalar.activation(sbc[d][:], ones[:], ACTF.Copy,
                                         scale=s_sb[:, d:d + 1])
                    nc.tensor.matmul(psw[:], sbc[d][:], phixy[i][d][:],
                                     start=(d == 0), stop=(d == DT - 1))
                wxy = sp.tile([128, W2T], F16, tag=f"wxy_{i}", name=f"wxy_{i}")
                nc.scalar.activation(wxy[:], psw[:], ACTF.Copy, bias=c0)

                # out^T = x^T * wx + y^T * wy
                osb = op.tile([128, DT * N], F16, tag=f"osb_{i}", name=f"osb_{i}")
                for d in range(DT):
                    xs = xg[:, d * W2T + i * N: d * W2T + (i + 1) * N]
                    ys = yg[:, d * W2T + i * N: d * W2T + (i + 1) * N]
                    tmp = op.tile([128, N], F16, tag="tmp", name="tmp")
                    nc.vector.tensor_tensor(osb[:, d * N:(d + 1) * N], xs,
                                            wxy[:, 0:N], ALU.mult)
                    nc.vector.tensor_tensor(tmp[:], ys, wxy[:, N:W2T], ALU.mult)
                    nc.gpsimd.tensor_tensor(osb[:, d * N:(d + 1) * N],
                                            osb[:, d * N:(d + 1) * N],
                                            tmp[:], ALU.add)
                nc.sync.dma_start(out=out_d[G * g + i], in_=osb[:])

        # Software-pipeline by one group: PE runs group g's dense main
        # matmuls while group g-1's DVE/ACT reduction chain + matvec drain.
        prev = None
        for g in range(NG):
            cur = emit_mains(g)
            if prev is not None:
                emit_tail(g - 1, *prev)
            prev = cur
        emit_tail(NG - 1, *prev)

    _split_multi_waits(nc)
    return nc


def _split_multi_waits(nc):
    """This walrus build accepts at most ONE sync-wait command per TPB
    instruction; the Tile scheduler happily emits several. Hoist all but the
    last wait of each instruction onto same-engine EventSemaphore ops placed
    immediately before it (engine program order is the within-block
    subsequence, so this preserves semantics)."""
    import json
    data = json.loads(nc.to_json_bytes())
    n = 0
    for fn in data["functions"]:
        for blk in fn["blocks"]:
            out = []
            for inst in blk["instructions"]:
                si = inst.get("sync_info")
                ow = (si or {}).get("on_wait") or []
                if len(ow) > 1:
                    for w in ow[:-1]:
                        n += 1
                        out.append({
                            "name": f"eswait_{n}",
                            "opcode": "EventSemaphore",
                            "engine": inst["engine"],
                            "ins": [],
                            "outs": [],
                            "sync_info": {"on_wait": [w], "on_update": []},
                        })
                    si["on_wait"] = [ow[-1]]
                out.append(inst)
            blk["instructions"] = out
    nc.m = mybir.module_from_json_bytes(json.dumps(data).encode())
    return nc


def prep_host(inputs: dict):
    x = np.ascontiguousarray(np.asarray(inputs["x"], dtype=np.float32))
    y = np.ascontiguousarray(np.asarray(inputs["y"], dtype=np.float32))
    W1 = np.asarray(inputs["W1"], dtype=np.float32)
    W2 = np.asarray(inputs["W2"], dtype=np.float32)
    g1 = np.asarray(inputs["g1"], dtype=np.float32)
    g2 = np.asarray(inputs["g2"], dtype=np.float32)
    b1 = np.asarray(inputs["b1"], dtype=np.float32)
    b2 = np.asarray(inputs["b2"], dtype=np.float32)
    be1 = np.asarray(inputs["be1"], dtype=np.float32)
    be2 = np.asarray(inputs["be2"], dtype=np.float32)
    W3 = np.asarray(inputs["W3"], dtype=np.float32)
    b3 = np.asarray(inputs["b3"], dtype=np.float32)
    W4 = np.asarray(inputs["W4"], dtype=np.float32)
    b4 = np.asarray(inputs["b4"], dtype=np.float32)
    W5 = np.asarray(inputs["W5"], dtype=np.float32)
    b5 = np.asarray(inputs["b5"], dtype=np.float32)

    W1p = np.ascontiguousarray(W1 * g1[None, :])
    W2p = np.ascontiguousarray(W2 * g2[None, :])
    b1p = b1 * g1 + be1
    b2p = b2 * g2 + be2
    W5a, W5b = W5[:C, 0], W5[C:, 0]
    u3 = (W3 @ W5a).astype(np.float32)
    u4 = (W4 @ W5b).astype(np.float32)
    z = (u4[:2 * N] + u4[2 * N:]).astype(np.float32)
    c0 = float(b3 @ W5a + b4 @ W5b + b5[0])

    # [B,N,C] -> per-core groups [M, NG, 128, DT*392] with [x_a|x_b] 392-blocks
    def pack(a):
        at = a.transpose(0, 2, 1).reshape(NCORES, S, DT, 128, N)
        pair = at.reshape(NCORES, NG, G, DT, 128, N)
        gg = np.concatenate([pair[:, :, 0], pair[:, :, 1]], axis=-1)  # [M,NG,DT,128,392]
        return np.ascontiguousarray(
            gg.transpose(0, 1, 3, 2, 4).reshape(NCORES, NG, 128, DT * W2T)
            .astype(np.float16))

    XG, YG = pack(x), pack(y)
    W1p = W1p.astype(np.float16)
    W2p = W2p.astype(np.float16)
    zb = np.ascontiguousarray(
        np.broadcast_to(z[None, :], (128, W2T))).astype(np.float16)
    u3t = np.ascontiguousarray(u3.reshape(DT, 128).T)
    b1t = np.ascontiguousarray(b1p.reshape(DT, 128).T)
    b2t = np.ascontiguousarray(b2p.reshape(DT, 128).T)

    in_maps = []
    for cidx in range(NCORES):
        in_maps.append({
            "xg": XG[cidx], "yg": YG[cidx], "w1": W1p, "w2": W2p,
            "zb": zb, "u3": u3t, "b1": b1t, "b2": b2t,
        })
    return in_maps, c0, x, y


def unpack_out(results) -> np.ndarray:
    outs = []
    for cidx in range(NCORES):
        o = np.asarray(results[cidx]["out"]).astype(np.float32)  # [S, 128, DT*N]
        o = o.reshape(S, 128, DT, N).transpose(0, 2, 1, 3).reshape(S, C, N)
        outs.append(o.transpose(0, 2, 1))     # [S, N, C]
    return np.ascontiguousarray(np.concatenate(outs, axis=0))


def kernel(**inputs) -> np.ndarray:
    in_maps, c0, _, _ = prep_host(inputs)
    nc = build_bass(c0)
    res = run_bass_kernel_spmd(nc, in_maps, list(range(NCORES)))
    return unpack_out(res.results)



# revision 12
# speedup vs baseline: 2.6649x; 2.6649x over previous
"""Trainium2 Bass kernel for the CRA relation module.

Math: the reference computes, per sample,
    phi_x = relu((x@W1+b1)*g1+be1), phi_y likewise,  cat_phi = [phi_x; phi_y]
    A = cat_phi cat_phi^T (symmetric!),  R = [A | A^T] = [A | A]
    W = (cat_phi@W3+b3)@W5a + (R@W4+b4)@W5b + b5
    out = x * W[:196] + y * W[196:]
Because A is symmetric and everything after A is linear into a scalar per
token, the relation pipeline collapses to per-sample matvecs:
    u3 = W3@W5a, u4 = W4@W5b, z = u4[:392]+u4[392:], c0 = b3@W5a+b4@W5b+b5
    s  = u3 + phi_x^T z[:196] + phi_y^T z[196:]          (768-vector)
    out = x*(phi_x@s + c0) + y*(phi_y@s + c0)
The device only computes the two 768x768 "1x1 conv" matmuls (the dominant
cost), one fused multiply-reduce, and one matvec per stream. Everything is
data-parallel over the batch: 16 samples per core on 8 cores.

Device layout: feature-major ("transposed") so the contraction dim (cin)
sits on SBUF partitions. The host packs x into [group, 128, 6*392] where
each 392-column block holds [x_a | x_b] for one cin tile of two samples,
so one DMA per group is fully contiguous.
"""

import numpy as np
from contextlib import ExitStack

import concourse.bass as bass
import concourse.tile as tile
import concourse.mybir as mybir
from concourse.bass_utils import run_bass_kernel_spmd

F32 = mybir.dt.float32
F16 = mybir.dt.float16
ALU = mybir.AluOpType
ACTF = mybir.ActivationFunctionType

B, N, C = 128, 196, 768
NCORES = 8
S = B // NCORES          # 16 samples per core
G = 2                    # samples per weight pass (moving N = 392 <= 512 fp32)
NG = S // G              # 8 groups per core
DT = C // 128            # 6 feature tiles
W2T = 2 * N              # 392


def build_bass(c0: float, for_sim: bool = False) -> bass.Bass:
    nc = bass.Bass()
    xg_d = nc.declare_dram_parameter("xg", [NG, 128, DT * W2T], F16, isOutput=False)
    yg_d = nc.declare_dram_parameter("yg", [NG, 128, DT * W2T], F16, isOutput=False)
    w1_d = nc.declare_dram_parameter("w1", [C, C], F16, isOutput=False)
    w2_d = nc.declare_dram_parameter("w2", [C, C], F16, isOutput=False)
    zb_d = nc.declare_dram_parameter("zb", [128, W2T], F16, isOutput=False)
    u3_d = nc.declare_dram_parameter("u3", [128, DT], F32, isOutput=False)
    b1_d = nc.declare_dram_parameter("b1", [128, DT], F32, isOutput=False)
    b2_d = nc.declare_dram_parameter("b2", [128, DT], F32, isOutput=False)
    out_d = nc.declare_dram_parameter("out", [S, 128, DT * N], F16, isOutput=True)

    with tile.TileContext(nc) as tc, ExitStack() as ctx:
        const = ctx.enter_context(tc.tile_pool(name="const", bufs=1))

        w1_sb, w2_sb = [], []
        for k in range(DT):
            t1 = const.tile([128, C], F16, tag=f"w1_{k}")
            nc.sync.dma_start(out=t1[:], in_=w1_d[k * 128:(k + 1) * 128, :])
            w1_sb.append(t1)
            t2 = const.tile([128, C], F16, tag=f"w2_{k}")
            nc.sync.dma_start(out=t2[:], in_=w2_d[k * 128:(k + 1) * 128, :])
            w2_sb.append(t2)

        zb = const.tile([128, W2T], F16, tag="zb")
        nc.sync.dma_start(out=zb[:], in_=zb_d[:, :])
        u3 = const.tile([128, DT], F32, tag="u3")
        nc.sync.dma_start(out=u3[:], in_=u3_d[:, :])
        b1t = const.tile([128, DT], F32, tag="b1")
        nc.sync.dma_start(out=b1t[:], in_=b1_d[:, :])
        b2t = const.tile([128, DT], F32, tag="b2")
        nc.sync.dma_start(out=b2t[:], in_=b2_d[:, :])
        ones = const.tile([128, 128], F16, tag="ones")
        nc.vector.memset(ones[:], 1.0)
        # Absorb the bias-tile DMA deps into ACT program order now, so the
        # relu evictions later only ever wait on the PE semaphore (the ISA
        # Activation descriptor holds a single sync-wait).
        warm1 = const.tile([128, 1], F32, tag="warm1")
        warm2 = const.tile([128, 1], F32, tag="warm2")
        nc.scalar.activation(warm1[:], b1t[:, 0:1], ACTF.Copy)
        nc.scalar.activation(warm2[:], b2t[:, 0:1], ACTF.Copy)

        xin = ctx.enter_context(tc.tile_pool(name="xin", bufs=3))
        phip = ctx.enter_context(tc.tile_pool(name="phi", bufs=3))
        sp = ctx.enter_context(tc.tile_pool(name="sp", bufs=3))
        op = ctx.enter_context(tc.tile_pool(name="op", bufs=2))
        ps = ctx.enter_context(tc.tile_pool(name="ps", bufs=2, space="PSUM"))

        def emit_mains(g):
            xg = xin.tile([128, DT * W2T], F16, tag="xg", name="xg")
            yg = xin.tile([128, DT * W2T], F16, tag="yg", name="yg")
            nc.sync.dma_start(out=xg[:], in_=xg_d[g])
            nc.sync.dma_start(out=yg[:], in_=yg_d[g])
            # phixy[i][d]: [128, 392] = [phi_x | phi_y] of sample (2g+i), tile d
            phixy = [[phip.tile([128, W2T], F16, tag=f"phi_{i}_{d}",
                                name=f"phi_{i}_{d}") for d in range(DT)]
                     for i in range(G)]
            for d in range(DT):
                psx = ps.tile([128, W2T], F32, tag="psx", name="psx", bufs=3)
                psy = ps.tile([128, W2T], F32, tag="psy", name="psy", bufs=3)
                for k in range(DT):
                    nc.tensor.matmul(
                        psx[:], w1_sb[k][:, d * 128:(d + 1) * 128],
                        xg[:, k * W2T:(k + 1) * W2T],
                        start=(k == 0), stop=(k == DT - 1))
                for k in range(DT):
                    nc.tensor.matmul(
                        psy[:], w2_sb[k][:, d * 128:(d + 1) * 128],
                        yg[:, k * W2T:(k + 1) * W2T],
                        start=(k == 0), stop=(k == DT - 1))
                for i in range(G):
                    nc.scalar.activation(phixy[i][d][:, 0:N], psx[:, i * N:(i + 1) * N],
                                         ACTF.Relu, bias=b1t[:, d:d + 1])
                    nc.scalar.activation(phixy[i][d][:, N:W2T], psy[:, i * N:(i + 1) * N],
                                         ACTF.Relu, bias=b2t[:, d:d + 1])
            return xg, yg, phixy

        def emit_tail(g, xg, yg, phixy):
            for i in range(G):
                # s = u3 + phi_x^T zx + phi_y^T zy  (per feature tile d)
                t_sb = sp.tile([128, DT], F32, tag=f"t_{i}", name=f"t_{i}")
                s_sb = sp.tile([128, DT], F32, tag=f"s_{i}", name=f"s_{i}")
                for d in range(DT):
                    scr = sp.tile([128, W2T], F16, tag="ttr_scr", name="scr")
                    nc.vector.scalar_tensor_tensor(
                        out=scr[:], in0=phixy[i][d][:], scalar=1.0, in1=zb[:],
                        op0=ALU.mult, op1=ALU.mult,
                        accum_out=t_sb[:, d:d + 1])
                    nc.vector.tensor_scalar_add(
                        s_sb[:, d:d + 1], t_sb[:, d:d + 1], u3[:, d:d + 1])
                # wxy[tok] = phi @ s on all 128 partitions via bcast lhsT
                psw = ps.tile([128, W2T], F32, tag="psw", name="psw")
                sbc = [sp.tile([128, 128], F16, tag=f"sbc_{d}", name=f"sbc_{d}")
                       for d in range(DT)]
                for d in range(DT):
                    nc.scalar.activation(sbc[d][:], ones[:], ACTF.Copy,
                                         scale=s_sb[:, d:d + 1])
                    nc.tensor.matmul(psw[:], sbc[d][:], phixy[i][d][:],
                                     start=(d == 0), stop=(d == DT - 1))
                wxy = sp.tile([128, W2T], F16, tag=f"wxy_{i}", name=f"wxy_{i}")
                nc.scalar.activation(wxy[:], psw[:], ACTF.Copy, bias=c0)

                # out^T = x^T * wx + y^T * wy
                osb = op.tile([128, DT * N], F16, tag=f"osb_{i}", name=f"osb_{i}")
                for d in range(DT):
                    xs = xg[:, d * W2T + i * N: d * W2T + (i + 1) * N]
                    ys = yg[:, d * W2T + i * N: d * W2T + (i + 1) * N]
                    tmp = op.tile([128, N], F16, tag="tmp", name="tmp")
                    nc.vector.tensor_tensor(osb[:, d * N:(d + 1) * N], xs,
                                            wxy[:, 0:N], ALU.mult)
                    nc.vector.tensor_tensor(tmp[:], ys, wxy[:, N:W2T], ALU.mult)
                    nc.gpsimd.tensor_tensor(osb[:, d * N:(d + 1) * N],
                                            osb[:, d * N:(d + 1) * N],
                                            tmp[:], ALU.add)
                nc.sync.dma_start(out=out_d[G * g + i], in_=osb[:])

        # Software-pipeline by one group: PE runs group g's dense main
        # matmuls while group g-1's DVE/ACT reduction chain + matvec drain.
        prev = None
        for g in range(NG):
            cur = emit_mains(g)
            if prev is not None:
                emit_tail(g - 1, *prev)
            prev = cur
        emit_tail(NG - 1, *prev)

    if not for_sim:
        _split_multi_waits(nc)
    return nc


def _split_multi_waits(nc):
    """This walrus build accepts at most ONE sync-wait command per TPB
    instruction; the Tile scheduler happily emits several. Hoist all but the
    last wait of each instruction onto same-engine EventSemaphore ops placed
    immediately before it (engine program order is the within-block
    subsequence, so this preserves semantics)."""
    import json
    data = json.loads(nc.to_json_bytes())
    n = 0
    for fn in data["functions"]:
        for blk in fn["blocks"]:
            out = []
            for inst in blk["instructions"]:
                si = inst.get("sync_info")
                ow = (si or {}).get("on_wait") or []
                if len(ow) > 1:
                    for w in ow[:-1]:
                        n += 1
                        out.append({
                            "name": f"eswait_{n}",
                            "opcode": "EventSemaphore",
                            "engine": inst["engine"],
                            "ins": [],
                            "outs": [],
                            "sync_info": {"on_wait": [w], "on_update": []},
                        })
                    si["on_wait"] = [ow[-1]]
                out.append(inst)
            blk["instructions"] = out
    nc.m = mybir.module_from_json_bytes(json.dumps(data).encode())
    return nc


def prep_host(inputs: dict):
    x = np.ascontiguousarray(np.asarray(inputs["x"], dtype=np.float32))
    y = np.ascontiguousarray(np.asarray(inputs["y"], dtype=np.float32))
    W1 = np.asarray(inputs["W1"], dtype=np.float32)
    W2 = np.asarray(inputs["W2"], dtype=np.float32)
    g1 = np.asarray(inputs["g1"], dtype=np.float32)
    g2 = np.asarray(inputs["g2"], dtype=np.float32)
    b1 = np.asarray(inputs["b1"], dtype=np.float32)
    b2 = np.asarray(inputs["b2"], dtype=np.float32)
    be1 = np.asarray(inputs["be1"], dtype=np.float32)
    be2 = np.asarray(inputs["be2"], dtype=np.float32)
    W3 = np.asarray(inputs["W3"], dtype=np.float32)
    b3 = np.asarray(inputs["b3"], dtype=np.float32)
    W4 = np.asarray(inputs["W4"], dtype=np.float32)
    b4 = np.asarray(inputs["b4"], dtype=np.float32)
    W5 = np.asarray(inputs["W5"], dtype=np.float32)
    b5 = np.asarray(inputs["b5"], dtype=np.float32)

    W1p = np.ascontiguousarray(W1 * g1[None, :])
    W2p = np.ascontiguousarray(W2 * g2[None, :])
    b1p = b1 * g1 + be1
    b2p = b2 * g2 + be2
    W5a, W5b = W5[:C, 0], W5[C:, 0]
    u3 = (W3 @ W5a).astype(np.float32)
    u4 = (W4 @ W5b).astype(np.float32)
    z = (u4[:2 * N] + u4[2 * N:]).astype(np.float32)
    c0 = float(b3 @ W5a + b4 @ W5b + b5[0])

    # [B,N,C] -> per-core groups [M, NG, 128, DT*392] with [x_a|x_b] 392-blocks
    def pack(a):
        at = a.transpose(0, 2, 1).reshape(NCORES, S, DT, 128, N)
        pair = at.reshape(NCORES, NG, G, DT, 128, N)
        gg = np.concatenate([pair[:, :, 0], pair[:, :, 1]], axis=-1)  # [M,NG,DT,128,392]
        return np.ascontiguousarray(
            gg.transpose(0, 1, 3, 2, 4).reshape(NCORES, NG, 128, DT * W2T)
            .astype(np.float16))

    XG, YG = pack(x), pack(y)
    W1p = W1p.astype(np.float16)
    W2p = W2p.astype(np.float16)
    zb = np.ascontiguousarray(
        np.broadcast_to(z[None, :], (128, W2T))).astype(np.float16)
    u3t = np.ascontiguousarray(u3.reshape(DT, 128).T)
    b1t = np.ascontiguousarray(b1p.reshape(DT, 128).T)
    b2t = np.ascontiguousarray(b2p.reshape(DT, 128).T)

    in_maps = []
    for cidx in range(NCORES):
        in_maps.append({
            "xg": XG[cidx], "yg": YG[cidx], "w1": W1p, "w2": W2p,
            "zb": zb, "u3": u3t, "b1": b1t, "b2": b2t,
        })
    return in_maps, c0, x, y


def unpack_out(results) -> np.ndarray:
    outs = []
    for cidx in range(NCORES):
        o = np.asarray(results[cidx]["out"]).astype(np.float32)  # [S, 128, DT*N]
        o = o.reshape(S, 128, DT, N).transpose(0, 2, 1, 3).reshape(S, C, N)
        outs.append(o.transpose(0, 2, 1))     # [S, N, C]
    return np.ascontiguousarray(np.concatenate(outs, axis=0))


def kernel(**inputs) -> np.ndarray:
    in_maps, c0, _, _ = prep_host(inputs)
    nc = build_bass(c0)
    res = run_bass_kernel_spmd(nc, in_maps, list(range(NCORES)))
    return unpack_out(res.results)

